# revision 1
# baseline (speedup 1.0000x reference)
"""Trainium2 Bass kernel for nn_DescriptorModuleSpecies (gnn_message_passing).

Sharding: data-parallel, one snapshot per NeuronCore (8 cores).

Algorithmic core (exact algebra of the reference):
    D[n] = Q[n]^T @ Q[n][:, :16],   Q[n] = sum_m r_tilde(n,m) ⊗ G(s(n,m), pair)
The species-pair MLPs (es/fs) and en1/en2 are folded on the host into an
exact piecewise-linear basis in s per species-pair class:
    G(s; class) = sum_beta phi_beta(s) * W3''[beta, :]      (W ~= 54 basis fns)
Per-edge basis planes cost one fused DVE/ACT op each; per-atom moments
Phi[d, beta] = sum_m r_tilde_d * phi_beta are computed with one small PE
matmul per atom-pair column (contraction over the 128 edge rows of the
(2 atoms x 64 slots) layout), then Q = Phi @ W3'' and D via per-atom
broadcast multiplies on DVE.

Neighbor gather: gpsimd ap_gather from an SBUF-resident interleaved table
(partition p holds component p%4 of (x, y, z, type)), per-Q7-core index
streams prepared on the host (pure index-layout preprocessing), followed by
SBUF->SBUF DMAs splitting component rows into edge planes.
"""

import sys

import numpy as np

try:
    import concourse.bass as bass  # noqa: F401
except Exception:  # pragma: no cover
    sys.path.insert(0, "/opt/trn_rl_repo")

import concourse.bass as bass
import concourse.bacc as bacc
import concourse.mybir as mybir
from concourse.bass_utils import run_bass_kernel_spmd
from concourse.tile import TileContext

F32 = mybir.dt.float32
I32 = mybir.dt.int32
I16 = mybir.dt.int16
AF = mybir.ActivationFunctionType
ALU = mybir.AluOpType

S, N, M = 8, 4096, 64
L = 20.0
JTOT = N // 2              # 2048 atom-pair columns
NCHUNK = 8
JC = JTOT // NCHUNK        # 256 cols per chunk
NI = 16 * JC               # ap_gather num_idxs per Q7 core per chunk
NCORES = 8
SUBJ = 128                 # moment sub-chunk (j columns per bas tile)

CLASSES = [(0, 0), (0, 1), (1, 1)]   # pair (0,1) == (1,0) exactly (symmetrized)


def _mlp_np(x, params):
    n = len(params)
    for i, (w, b) in enumerate(params):
        x = x @ w + b
        if i < n - 1:
            x = np.maximum(x, 0.0)
    return x


def _fold_weights(ws):
    """Exact PL basis for h2(s; class) folded with en3 into W3''.

    Returns (basis, w3pp): basis is a list of ("one"|"lin"|"relu", cls, knot);
    w3pp [W, 32] f32 with G_edge = sum_beta basis_beta * w3pp[beta]."""
    es = [(ws["es1_w"], ws["es1_b"]), (ws["es2_w"], ws["es2_b"])]
    fs = [(ws["fs1_w"], ws["fs1_b"]), (ws["fs2_w"], ws["fs2_b"])]
    W1, b1 = ws["en1_w"].astype(np.float64), ws["en1_b"].astype(np.float64)
    W2, b2 = ws["en2_w"].astype(np.float64), ws["en2_b"].astype(np.float64)
    W3, b3 = ws["en3_w"].astype(np.float64), ws["en3_b"].astype(np.float64)

    basis, psis = [], []
    for ci, (a, b) in enumerate(CLASSES):
        pair = np.array([[a, b]], dtype=np.float32)
        td = _mlp_np(_mlp_np(pair, es) + _mlp_np(pair[:, ::-1], es), fs)[0]
        td = td.astype(np.float64)
        U = td @ W1                                   # [8]

        def h2_of(s):
            h1 = np.maximum(np.outer(s, U) + b1[None, :], 0.0)
            return np.maximum(h1 @ W2 + b2[None, :], 0.0)

        kn1 = sorted(float(-b1[c] / U[c]) for c in range(8)
                     if U[c] != 0.0 and -b1[c] / U[c] > 0.0)
        segpts = [0.0] + kn1
        cross = set()
        for i in range(len(segpts)):
            lo = segpts[i]
            hi = segpts[i + 1] if i + 1 < len(segpts) else None
            mid = (lo + hi) / 2 if hi is not None else lo + 1.0
            act = (mid * U + b1) > 0
            z_lo = np.maximum(lo * U + b1, 0.0) @ W2 + b2
            slope = (U * act) @ W2
            for f in range(16):
                if slope[f] == 0.0:
                    continue
                t = lo - z_lo[f] / slope[f]
                if t > lo and (hi is None or t < hi) and t > 0.0:
                    cross.add(float(t))
        knots = sorted(set(kn1) | cross)

        def seg_slope(lo, hi):
            mid = (lo + hi) / 2 if hi is not None else lo + 1.0
            act1 = (mid * U + b1) > 0
            z_mid = np.maximum(mid * U + b1, 0.0) @ W2 + b2
            return ((U * act1) @ W2) * (z_mid > 0)

        alpha = h2_of(np.array([0.0]))[0]
        bounds = knots + [None]
        slopes = [seg_slope(0.0 if i == 0 else knots[i - 1], bounds[i])
                  for i in range(len(knots) + 1)]
        basis.append(("one", ci, 0.0)); psis.append(alpha)
        basis.append(("lin", ci, 0.0)); psis.append(slopes[0])
        for i, t in enumerate(knots):
            basis.append(("relu", ci, float(t)))
            psis.append(slopes[i + 1] - slopes[i])

    Psi = np.stack(psis, 0)
    w3pp = Psi @ W3
    for i, (kind, ci, t) in enumerate(basis):
        if kind == "one":
            w3pp[i] += b3
    return basis, w3pp.astype(np.float32)


def _verify_fold(ws, basis, w3pp):
    es = [(ws["es1_w"], ws["es1_b"]), (ws["es2_w"], ws["es2_b"])]
    fs = [(ws["fs1_w"], ws["fs1_b"]), (ws["fs2_w"], ws["fs2_b"])]
    rng = np.random.default_rng(0)
    sv = np.concatenate([rng.uniform(0, 5, 64), rng.uniform(0, 1000, 32), [0.0]])
    for ci, (a, b) in enumerate(CLASSES):
        pair = np.array([[a, b]], dtype=np.float32)
        td = _mlp_np(_mlp_np(pair, es) + _mlp_np(pair[:, ::-1], es), fs)[0]
        st = sv[:, None] * td[None, :].astype(np.float64)
        G = _mlp_np(st, [(ws["en1_w"], ws["en1_b"]), (ws["en2_w"], ws["en2_b"]),
                         (ws["en3_w"], ws["en3_b"])])
        vals = np.zeros((len(sv), len(basis)))
        for i, (kind, cc, t) in enumerate(basis):
            if cc != ci:
                continue
            vals[:, i] = 1.0 if kind == "one" else (sv if kind == "lin"
                                                    else np.maximum(sv - t, 0.0))
        Gb = vals @ w3pp.astype(np.float64)
        err = np.abs(Gb - G).max() / (np.abs(G).max() + 1e-9)
        assert err < 1e-4, f"basis fold mismatch class {ci}: rel {err}"


def _reg_consts(nc, vals):
    for v in vals:
        key = (F32, float(v))
        if key in nc.const_aps.aps:
            continue
        t = nc.alloc_sbuf_tensor(f"constf32_{len(nc.const_aps.aps)}", [128, 1], F32)
        nc.gpsimd.memset(t.ap(), float(v))
        nc.const_aps.aps[key] = t.ap()
    nc.all_engine_barrier()


def _build_program(basis):
    Wb = len(basis)
    assert Wb <= 128
    # engine split for basis relu planes: alternate DVE / ACT
    act_knots = sorted({t for k, c, t in basis if k == "relu"})

    nc = bacc.Bacc("TRN2", target_bir_lowering=False, debug=False,
                   num_devices=NCORES)
    _reg_consts(nc, [0.0, 1e-12, float(np.pi)] + [-t for t in act_knots])

    table = nc.dram_tensor("table", [128, N], F32, kind="ExternalInput")
    idxw = nc.dram_tensor("idxw", [128, JTOT], I16, kind="ExternalInput")
    nqd = nc.dram_tensor("nq", [128, JTOT], I32, kind="ExternalInput")
    xi = nc.dram_tensor("xi", [128, JTOT], F32, kind="ExternalInput")
    yi = nc.dram_tensor("yi", [128, JTOT], F32, kind="ExternalInput")
    zi = nc.dram_tensor("zi", [128, JTOT], F32, kind="ExternalInput")
    ai = nc.dram_tensor("ai", [128, JTOT], F32, kind="ExternalInput")
    w3t = nc.dram_tensor("w3pp", [Wb, 32], F32, kind="ExternalInput")
    dout = nc.dram_tensor("dout", [N, 512], F32, kind="ExternalOutput")

    with TileContext(nc) as tc:
        with (
            tc.tile_pool(name="persist", bufs=1) as pp,
            tc.tile_pool(name="work", bufs=2) as wp,
            tc.tile_pool(name="bas", bufs=1) as bp,
            tc.tile_pool(name="psum", bufs=4, space="PSUM") as psp,
            tc.tile_pool(name="qpsum", bufs=4, space="PSUM") as qsp,
        ):
            tab = pp.tile([128, N], F32)
            nc.sync.dma_start(tab[:], table[:])
            w3s = pp.tile([Wb, 32], F32)
            nc.sync.dma_start(w3s[:], w3t[:])
            qt = pp.tile([128, 128 * 32], F32)     # [(8j16+4q+d), 32*grp + g]
            q2 = pp.tile([128, 4096], F32)         # [atom%128, 128*t + 32*d + g]

            for c in range(NCHUNK):
                j0 = c * JC
                idx = wp.tile([128, JC], I16, tag="idx")
                nc.sync.dma_start(idx[:], idxw[:, j0:j0 + JC])
                nqc = wp.tile([128, JC], I32, tag="nqc")
                nc.sync.dma_start(nqc[:], nqd[:, j0:j0 + JC])
                xic = wp.tile([128, JC], F32, name="xic", tag="xic")
                nc.sync.dma_start(xic[:], xi[:, j0:j0 + JC])
                yic = wp.tile([128, JC], F32, name="yic", tag="yic")
                nc.sync.dma_start(yic[:], yi[:, j0:j0 + JC])
                zic = wp.tile([128, JC], F32, name="zic", tag="zic")
                nc.sync.dma_start(zic[:], zi[:, j0:j0 + JC])
                aicp = wp.tile([128, JC], F32, name="aicp", tag="aicp")
                nc.sync.dma_start(aicp[:], ai[:, j0:j0 + JC])
                gx = wp.tile([128, NI], F32, name="gx", tag="gx", bufs=1)
                nc.gpsimd.ap_gather(out_ap=gx[:], in_ap=tab[:], idxs_ap=idx[:],
                                    channels=128, num_elems=N, d=1, num_idxs=NI)

                XJ = wp.tile([128, JC], F32, tag="XJ")
                YJ = wp.tile([128, JC], F32, tag="YJ")
                ZJ = wp.tile([128, JC], F32, tag="ZJ")
                BJ = wp.tile([128, JC], F32, tag="BJ")
                for comp, dst in ((0, XJ), (1, YJ), (2, ZJ), (3, BJ)):
                    for k in range(NCORES):
                        src = gx[16 * k + comp:16 * k + comp + 1, :]
                        src3 = src.rearrange("p (s j) -> p s j", s=16)
                        nc.sync.dma_start(dst[16 * k:16 * k + 16, :], src3)

                def plane(tag):
                    return wp.tile([128, JC], F32, name=tag, tag=tag)

                ux, uy, uz = plane("ux"), plane("uy"), plane("uz")
                nc.vector.tensor_tensor(out=ux[:], in0=XJ[:], in1=xic[:], op=ALU.subtract)
                nc.vector.tensor_tensor(out=uy[:], in0=YJ[:], in1=yic[:], op=ALU.subtract)
                nc.vector.tensor_tensor(out=uz[:], in0=ZJ[:], in1=zic[:], op=ALU.subtract)
                g1 = plane("g1"); g2 = plane("g2"); km = plane("km")
                for u_ in (ux, uy, uz):
                    nc.vector.tensor_scalar(out=g1[:], in0=u_[:], scalar1=10.0,
                                            scalar2=None, op0=ALU.is_gt)
                    nc.vector.tensor_scalar(out=g2[:], in0=u_[:], scalar1=-10.0,
                                            scalar2=None, op0=ALU.is_lt)
                    nc.vector.tensor_tensor(out=km[:], in0=g1[:], in1=g2[:], op=ALU.subtract)
                    nc.vector.tensor_scalar(out=km[:], in0=km[:], scalar1=L,
                                            scalar2=None, op0=ALU.mult)
                    nc.vector.tensor_tensor(out=u_[:], in0=u_[:], in1=km[:], op=ALU.subtract)
                sqx, sqy, sqz = plane("sqx"), plane("sqy"), plane("sqz")
                nc.scalar.activation(sqx[:], ux[:], AF.Square)
                nc.scalar.activation(sqy[:], uy[:], AF.Square)
                nc.scalar.activation(sqz[:], uz[:], AF.Square)
                r2 = plane("r2")
                nc.vector.tensor_tensor(out=r2[:], in0=sqx[:], in1=sqy[:], op=ALU.add)
                nc.vector.tensor_tensor(out=r2[:], in0=r2[:], in1=sqz[:], op=ALU.add)
                r = plane("r")
                nc.scalar.activation(r[:], r2[:], AF.Sqrt, bias=1e-12)
                invr = plane("invr")
                nc.vector.reciprocal(invr[:], r[:])
                rc = plane("rc")
                nc.vector.tensor_scalar(out=rc[:], in0=r[:], scalar1=2.0,
                                        scalar2=None, op0=ALU.max)
                nc.vector.tensor_scalar(out=rc[:], in0=rc[:], scalar1=6.0,
                                        scalar2=None, op0=ALU.min)
                csw = plane("csw")
                nc.scalar.activation(csw[:], rc[:], AF.Sin,
                                     scale=float(-np.pi / 4), bias=float(np.pi))
                swp = plane("swp")
                nc.vector.tensor_scalar(out=swp[:], in0=csw[:], scalar1=0.5,
                                        scalar2=0.5, op0=ALU.mult, op1=ALU.add)
                v = plane("v")
                nc.vector.tensor_scalar(out=v[:], in0=nqc[:], scalar1=0,
                                        scalar2=None, op0=ALU.is_ge)
                vir = plane("vir")
                nc.vector.tensor_tensor(out=vir[:], in0=v[:], in1=invr[:], op=ALU.mult)
                s2 = plane("s2")
                nc.vector.tensor_tensor(out=s2[:], in0=swp[:], in1=vir[:], op=ALU.mult)
                w0 = plane("w0")
                nc.vector.tensor_tensor(out=w0[:], in0=s2[:], in1=invr[:], op=ALU.mult)

                lt = wp.tile([128, JC, 8], F32, tag="lt")
                nc.vector.memset(lt[:], 0.0)
                nc.vector.tensor_copy(out=lt[0:64, :, 0], in_=s2[0:64, :])
                nc.vector.tensor_copy(out=lt[64:128, :, 4], in_=s2[64:128, :])
                for di, u_ in enumerate((ux, uy, uz)):
                    rij = plane("rij")
                    nc.vector.tensor_tensor(out=rij[:], in0=u_[:], in1=w0[:], op=ALU.mult)
                    nc.vector.tensor_copy(out=lt[0:64, :, 1 + di], in_=rij[0:64, :])
                    nc.vector.tensor_copy(out=lt[64:128, :, 5 + di], in_=rij[64:128, :])

                # class-masked s and one planes (classes 0,1,2)
                aic = aicp[:]
                scls, ocls = {}, {}
                sa1, sB = plane("sa1"), plane("sB")
                nc.vector.tensor_tensor(out=sa1[:], in0=s2[:], in1=aic, op=ALU.mult)
                nc.vector.tensor_tensor(out=sB[:], in0=s2[:], in1=BJ[:], op=ALU.mult)
                scls[2], u1s, u2s = plane("sc2"), plane("u1s"), plane("u2s")
                nc.vector.tensor_tensor(out=scls[2][:], in0=sa1[:], in1=BJ[:], op=ALU.mult)
                nc.vector.tensor_tensor(out=u1s[:], in0=sa1[:], in1=scls[2][:], op=ALU.subtract)
                nc.vector.tensor_tensor(out=u2s[:], in0=sB[:], in1=scls[2][:], op=ALU.subtract)
                scls[1], t3s, scls[0] = plane("sc1"), plane("t3s"), plane("sc0")
                nc.vector.tensor_tensor(out=scls[1][:], in0=u1s[:], in1=u2s[:], op=ALU.add)
                nc.vector.tensor_tensor(out=t3s[:], in0=s2[:], in1=sa1[:], op=ALU.subtract)
                nc.vector.tensor_tensor(out=scls[0][:], in0=t3s[:], in1=u2s[:], op=ALU.subtract)
                oa1, oB = plane("oa1"), plane("oB")
                nc.vector.tensor_tensor(out=oa1[:], in0=v[:], in1=aic, op=ALU.mult)
                nc.vector.tensor_tensor(out=oB[:], in0=v[:], in1=BJ[:], op=ALU.mult)
                ocls[2], u1o, u2o = plane("oc2"), plane("u1o"), plane("u2o")
                nc.vector.tensor_tensor(out=ocls[2][:], in0=oa1[:], in1=BJ[:], op=ALU.mult)
                nc.vector.tensor_tensor(out=u1o[:], in0=oa1[:], in1=ocls[2][:], op=ALU.subtract)
                nc.vector.tensor_tensor(out=u2o[:], in0=oB[:], in1=ocls[2][:], op=ALU.subtract)
                ocls[1], t3o, ocls[0] = plane("oc1"), plane("t3o"), plane("oc0")
                nc.vector.tensor_tensor(out=ocls[1][:], in0=u1o[:], in1=u2o[:], op=ALU.add)
                nc.vector.tensor_tensor(out=t3o[:], in0=v[:], in1=oa1[:], op=ALU.subtract)
                nc.vector.tensor_tensor(out=ocls[0][:], in0=t3o[:], in1=u2o[:], op=ALU.subtract)

                for sub in range(JC // SUBJ):
                    jlo = sub * SUBJ
                    bas = bp.tile([128, SUBJ, Wb], F32, tag="bas")
                    for bi, (kind, ci, t) in enumerate(basis):
                        if kind == "one":
                            nc.scalar.copy(bas[:, :, bi], ocls[ci][:, jlo:jlo + SUBJ])
                        elif kind == "lin":
                            nc.vector.tensor_copy(out=bas[:, :, bi],
                                                  in_=scls[ci][:, jlo:jlo + SUBJ])
                        else:
                            nc.scalar.activation(bas[:, :, bi],
                                                 scls[ci][:, jlo:jlo + SUBJ],
                                                 AF.Relu, bias=float(-t))
                    for grp in range(SUBJ // 16):
                        phps = psp.tile([128, 128], F32, tag="phps")
                        for jj in range(16):
                            j = jlo + grp * 16 + jj
                            nc.tensor.matmul(out=phps[:Wb, jj * 8:(jj + 1) * 8],
                                             lhsT=bas[:, j - jlo, :],
                                             rhs=lt[:, j, :],
                                             start=True, stop=True)
                        phi = wp.tile([128, 128], F32, tag="phi")
                        if grp % 2 == 0:
                            nc.scalar.copy(phi[:Wb, :], phps[:Wb, :])
                        else:
                            nc.vector.tensor_copy(out=phi[:Wb, :], in_=phps[:Wb, :])
                        g_abs = (c * JC + jlo) // 16 + grp
                        qps = qsp.tile([128, 32], F32, tag="qps")
                        nc.tensor.matmul(out=qps[:], lhsT=phi[:Wb, :], rhs=w3s[:],
                                         start=True, stop=True)
                        if grp % 2 == 0:
                            nc.vector.tensor_copy(
                                out=qt[:, g_abs * 32:(g_abs + 1) * 32], in_=qps[:])
                        else:
                            nc.scalar.copy(qt[:, g_abs * 32:(g_abs + 1) * 32], qps[:])

            # Q relayout: qt[8*j16+4*q+d, 32*gp+g] -> q2[32*(gp%4)+2*j16+q, 128*(gp//4)+32*d+g]
            qtv = qt[:].rearrange("p (gp g) -> p gp g", g=32)
            q2v = q2[:].rearrange("p (t d g) -> p t d g", d=4, g=32)
            for qq in range(2):
                for d in range(4):
                    for k4 in range(4):
                        src = qtv[4 * qq + d::8, k4::4, :]                 # [16, 32, 32]
                        dst = q2v[32 * k4 + qq:32 * k4 + qq + 31:2, :, d, :]
                        nc.sync.dma_start(dst, src)

            # D stage
            for t in range(32):
                acc = wp.tile([128, 512], F32, tag="dacc")
                tmp = wp.tile([128, 512], F32, tag="dtmp")
                for d in range(4):
                    off = 128 * t + 32 * d
                    qg = q2[:, off:off + 32]
                    in0 = qg.to_broadcast([128, 32, 16])
                    qk = q2[:, off:off + 16]
                    in1 = bass.AP(qk.tensor, qk.offset, [[4096, 128], [0, 32], [1, 16]])
                    dstv = (acc if d == 0 else tmp)[:].rearrange("p (g k) -> p g k", k=16)
                    nc.vector.tensor_tensor(out=dstv, in0=in0, in1=in1, op=ALU.mult)
                    if d > 0:
                        nc.vector.tensor_tensor(out=acc[:], in0=acc[:], in1=tmp[:], op=ALU.add)
                nc.sync.dma_start(dout[128 * t:128 * (t + 1), :], acc[:])

    nc.compile()
    return nc


def _prep_core(pos, types, neigh):
    comp = np.empty((4, N), np.float32)
    comp[0], comp[1], comp[2] = pos[:, 0], pos[:, 1], pos[:, 2]
    comp[3] = types.astype(np.float32)
    table = np.empty((128, N), np.float32)
    for p in range(128):
        table[p] = comp[p % 4]

    nv = neigh.reshape(JTOT, 2, M)
    nq = np.ascontiguousarray(nv.transpose(1, 2, 0).reshape(128, JTOT)).astype(np.int32)

    idxw = np.empty((128, JTOT), np.int16)
    nq_cl = np.maximum(nq, 0).astype(np.int16)
    for c in range(NCHUNK):
        blk = nq_cl[:, c * JC:(c + 1) * JC]
        for k in range(NCORES):
            stream = blk[16 * k:16 * k + 16, :].reshape(16 * JC)    # i = s*JC + j
            wrapped = stream.reshape(JC, 16).T                       # [p, cc]
            idxw[16 * k:16 * k + 16, c * JC:(c + 1) * JC] = wrapped

    par = pos.reshape(JTOT, 2, 3)
    def repl(x):  # [2, JTOT] -> [128, JTOT]
        return np.ascontiguousarray(
            np.broadcast_to(x[:, None, :], (2, M, JTOT)).reshape(128, JTOT)
        ).astype(np.float32)
    xi = repl(par[:, :, 0].T)
    yi = repl(par[:, :, 1].T)
    zi = repl(par[:, :, 2].T)
    ai = repl(types.reshape(JTOT, 2).T.astype(np.float32))
    return dict(table=table, idxw=idxw, nq=nq, xi=xi, yi=yi, zi=zi, ai=ai)


_CACHE = {}


def kernel(**inputs):
    inputs = {k: np.asarray(v) for k, v in inputs.items()}
    ws = {k: inputs[k].astype(np.float32) for k in
          ("es1_w", "es1_b", "es2_w", "es2_b", "fs1_w", "fs1_b", "fs2_w", "fs2_b",
           "en1_w", "en1_b", "en2_w", "en2_b", "en3_w", "en3_b")}
    key = hash(tuple(ws[k].tobytes() for k in sorted(ws)))
    if key not in _CACHE:
        basis, w3pp = _fold_weights(ws)
        _verify_fold(ws, basis, w3pp)
        nc = _build_program(basis)
        _CACHE[key] = (w3pp, nc)
    w3pp, nc = _CACHE[key]

    pos = inputs["inputs"].astype(np.float32)
    types = inputs["input_types"].astype(np.int64)
    neigh = inputs["neigh_list"].astype(np.int64)

    in_maps = []
    for s in range(S):
        m = _prep_core(pos[s], types[s], neigh[s])
        m["w3pp"] = w3pp
        in_maps.append(m)

    res = run_bass_kernel_spmd(nc, in_maps, core_ids=list(range(NCORES)))
    out = np.stack([r["dout"].reshape(N, 32, 16) for r in res.results], 0)
    return out.astype(np.float32)



# revision 2
# speedup vs baseline: 25815.1591x; 25815.1591x over previous
"""Trainium2 Bass kernel for nn_DescriptorModuleSpecies (gnn_message_passing).

Sharding: one snapshot per NeuronCore (8 cores), full inputs in / full out.

Algebra: D[n] = Q[n]^T @ Q[n][:, :16],  Q[n][d,g] = sum_m r_tilde_d(e) G_g(s_e).
G(s; class) is refit on a shared-knot PL basis phi = [1?, s, relu(s-t_k)] with
class folded into three moment weights {v, v(a+B), v a B} (T-matrix folded
into W3f host-side), so the device computes, per atom-pair column j:
    phi_psum[m*10+b, (nl,d)] = sum_{64 edge rows} bas[row, b] * LT[row, (m,d)]
(3 matmuls x 2 parity halves per column), then Q = W3f^T-contraction (PE),
then D as broadcasted products on DVE. Planes are fp16 (DVE 2x/4x modes);
geometry is f32. Min-image via fused (x+30) mod 20 - 10.
"""

import sys

import numpy as np

try:
    import concourse.bass as bass  # noqa: F401
except Exception:  # pragma: no cover
    sys.path.insert(0, "/opt/trn_rl_repo")

import concourse.bass as bass
import concourse.bacc as bacc
import concourse.mybir as mybir
from concourse.bass_utils import run_bass_kernel_spmd
from concourse.tile import TileContext

F32 = mybir.dt.float32
F16 = mybir.dt.float16
I16 = mybir.dt.int16
AF = mybir.ActivationFunctionType
ALU = mybir.AluOpType

S, N, M = 8, 4096, 64
L = 20.0
JTOT = N // 2               # 2048 atom-pair columns
NCHUNK = 4
JC = JTOT // NCHUNK         # 512 cols per chunk
NI = 16 * JC                # gather num_idxs per core per chunk
NCORES = 8

KNOTS = [0.09, 0.22, 0.44, 0.8, 2.9, 4.3, 5.4, 10.4]
WB = 2 + len(KNOTS)         # basis width: [v, s, relu x 8] = 10
WROWS = 3 * WB              # phi rows (3 m-weights stacked) = 30


# ---------------- host-side weight folding (shared-knot refit) --------------

def _mlp_np(x, params):
    n = len(params)
    for i, (w, b) in enumerate(params):
        x = x @ w + b
        if i < n - 1:
            x = np.maximum(x, 0.0)
    return x


def _exact_G(sv, ci, ws):
    es = [(ws["es1_w"], ws["es1_b"]), (ws["es2_w"], ws["es2_b"])]
    fs = [(ws["fs1_w"], ws["fs1_b"]), (ws["fs2_w"], ws["fs2_b"])]
    CL = [(0, 0), (0, 1), (1, 1)]
    a, b = CL[ci]
    pair = np.array([[a, b]], dtype=np.float64)
    td = _mlp_np(_mlp_np(pair, es) + _mlp_np(pair[:, ::-1], es), fs)[0]
    st = sv[:, None] * td[None, :]
    return _mlp_np(st, [(ws["en1_w"], ws["en1_b"]), (ws["en2_w"], ws["en2_b"]),
                        (ws["en3_w"], ws["en3_b"])])


def _fold_w3f(ws):
    """Fit G_c(s) ~= alpha[c]^T [1, s, relu(s-t)] and fold the class->m-weight
    transform:  e_c = T[c] . (m0, m1, m2) with m = (v, v(a+B), v a B)."""
    g1 = np.linspace(0.0, 0.6, 1200)
    g2 = np.linspace(0.6, 12.2, 1200)
    sv = np.concatenate([g1, g2])
    cols = [np.ones_like(sv), sv] + [np.maximum(sv - t, 0.0) for t in KNOTS]
    P = np.stack(cols, -1)
    lam = 1e-7
    PtP = P.T @ P + lam * np.eye(P.shape[1])
    alphas = []
    for ci in range(3):
        G = _exact_G(sv, ci, ws)
        A = np.linalg.solve(PtP, P.T @ G)
        alphas.append(A)
        resid = np.abs(P @ A - G).max()
        assert resid < 0.05, f"basis refit residual too large: {resid}"
    alpha = np.stack(alphas)                      # [3, WB, 32]
    T = np.array([[1.0, -1.0, 1.0],
                  [0.0, 1.0, -2.0],
                  [0.0, 0.0, 1.0]])
    W3f = np.einsum('cm,cjg->mjg', T, alpha)      # [3, WB, 32]
    # replicate per PE quadrant: w3rep[32q + b, 32m + g] = W3f[m, b, g]
    w3rep = np.zeros((128, 96), np.float16)
    for q in range(4):
        for m in range(3):
            w3rep[32 * q:32 * q + WB, 32 * m:32 * m + 32] = W3f[m]
    return w3rep


# ---------------------------- device program --------------------------------

def _build_program():
    nc = bacc.Bacc("TRN2", target_bir_lowering=False, debug=False,
                   num_devices=NCORES)
    # constants used by scalar-engine activations (bias/scale values)
    consts = [0.0, 1e-12, float(np.pi), 0.5] + [float(-t) for t in KNOTS]
    for v in consts:
        key = (F32, float(v))
        if key in nc.const_aps.aps:
            continue
        t = nc.alloc_sbuf_tensor(f"constf32_{len(nc.const_aps.aps)}", [128, 1], F32)
        nc.gpsimd.memset(t.ap(), float(v))
        nc.const_aps.aps[key] = t.ap()
    nc.all_engine_barrier()

    table = nc.dram_tensor("table", [128, N], F32, kind="ExternalInput")
    geo = nc.dram_tensor("geo", [128, 3 * JTOT], F32, kind="ExternalInput")
    aux = nc.dram_tensor("aux", [128, 3 * JTOT], I16, kind="ExternalInput")
    w3t = nc.dram_tensor("w3f", [128, 96], F16, kind="ExternalInput")
    dout = nc.dram_tensor("dout", [N, 512], F16, kind="ExternalOutput")

    with TileContext(nc) as tc:
        with (
            tc.tile_pool(name="persist", bufs=1) as pp,
            tc.tile_pool(name="geoin", bufs=2) as gp,
            tc.tile_pool(name="gxp", bufs=1) as xp,
            tc.tile_pool(name="edge", bufs=2) as ep,
            tc.tile_pool(name="scratch", bufs=1) as sp,
            tc.tile_pool(name="plane", bufs=2) as lp,
            tc.tile_pool(name="basp", bufs=2) as bpp,
            tc.tile_pool(name="grp", bufs=2) as grpp,
            tc.tile_pool(name="phips", bufs=2, space="PSUM") as psp,
            tc.tile_pool(name="q2ps", bufs=2, space="PSUM") as qsp,
        ):
            tab = pp.tile([128, N], F32)
            nc.sync.dma_start(tab[:], table[:])
            auxs = pp.tile([128, 3 * JTOT], I16)
            nc.sync.dma_start(auxs[:], aux[:])
            w3s = pp.tile([128, 96], F16)
            nc.sync.dma_start(w3s[:], w3t[:])

            vall = auxs[:, JTOT:2 * JTOT].bitcast(F16)
            aivall = auxs[:, 2 * JTOT:3 * JTOT].bitcast(F16)

            for c in range(NCHUNK):
                j0 = c * JC
                stage = gp.tile([128, 8 * 512], F16, tag="stage")
                geoc = gp.tile([128, 3 * JC], F32, tag="geoc")
                nc.sync.dma_start(geoc[:], geo[:, 3 * j0:3 * j0 + 3 * JC])
                vsl = vall[:, j0:j0 + JC]
                aivsl = aivall[:, j0:j0 + JC]

                gx = xp.tile([128, NI], F32, tag="gx")
                nc.gpsimd.ap_gather(out_ap=gx[:], in_ap=tab[:],
                                    idxs_ap=auxs[:, j0:j0 + JC],
                                    channels=128, num_elems=N, d=1, num_idxs=NI)
                # de-interleave components: rows {16k+comp} -> edge planes
                xyzt = ep.tile([128, 4 * JC], F32, tag="xyzt")
                for comp in range(4):
                    src = gx[comp::16, :]
                    src3 = src.rearrange("p (s j) -> p s j", s=16)
                    dst = xyzt[:, comp * JC:(comp + 1) * JC]
                    nc.sync.dma_start(dst, src3)
                XYZ = xyzt[:, 0:3 * JC]
                BJ = xyzt[:, 3 * JC:4 * JC]

                # ---- geometry (f32) ----
                t1 = sp.tile([128, 3 * JC], F32, tag="t1")
                nc.vector.tensor_tensor(out=t1[:], in0=XYZ, in1=geoc[:],
                                        op=ALU.subtract)
                # min image: wrap xj - xi back into [-10, 10] by one period
                nc.vector.add_range_wrap(t1[:], t1[:], shift=0.0,
                                         bound=10.0, period=L)
                sq = sp.tile([128, 3 * JC], F32, tag="sq")
                nc.scalar.activation(sq[:], t1[:], AF.Square)
                r2 = sp.tile([128, JC], F32, tag="r2")
                nc.vector.tensor_tensor(out=r2[:], in0=sq[:, 0:JC],
                                        in1=sq[:, JC:2 * JC], op=ALU.add)
                nc.gpsimd.tensor_tensor(out=r2[:], in0=r2[:],
                                        in1=sq[:, 2 * JC:3 * JC], op=ALU.add)
                r = sp.tile([128, JC], F32, tag="r")
                nc.scalar.activation(r[:], r2[:], AF.Sqrt, bias=1e-12)
                invr = sp.tile([128, JC], F32, tag="invr")
                nc.vector.reciprocal(invr[:], r[:])
                rc = sp.tile([128, JC], F32, tag="rc")
                nc.gpsimd.tensor_scalar(out=rc[:], in0=r[:], scalar1=2.0,
                                        scalar2=6.0, op0=ALU.max, op1=ALU.min)
                swp = sp.tile([128, JC], F32, tag="swp")
                nc.scalar.activation(swp[:], rc[:], AF.Sin,
                                     scale=float(-np.pi / 4), bias=float(np.pi))
                nc.scalar.activation(swp[:], swp[:], AF.Identity,
                                     bias=0.5, scale=0.5)
                vir = sp.tile([128, JC], F32, tag="vir")
                nc.vector.tensor_tensor(out=vir[:], in0=vsl, in1=invr[:],
                                        op=ALU.mult)

                # LT planes (fp16): [s2, rij x3, m1*(s2,rij), m2*(s2,rij)]
                lt = lp.tile([128, 12 * JC], F16, tag="lt")
                s2 = lt[:, 0:JC]
                nc.vector.tensor_tensor(out=s2, in0=swp[:], in1=vir[:],
                                        op=ALU.mult)
                w0 = sp.tile([128, JC], F32, tag="w0")
                nc.vector.tensor_tensor(out=w0[:], in0=s2, in1=invr[:],
                                        op=ALU.mult)
                w0b = bass.AP(w0.tensor, w0[:].offset,
                              [w0[:].ap[0], [0, 3], [1, JC]])
                rij3 = lt[:, JC:4 * JC].rearrange("p (c j) -> p c j", c=3)
                nc.vector.tensor_tensor(out=rij3, in0=t1[:].rearrange(
                    "p (c j) -> p c j", c=3), in1=w0b, op=ALU.mult)

                m1 = sp.tile([128, JC], F16, tag="m1")
                m2 = sp.tile([128, JC], F16, tag="m2")
                nc.gpsimd.tensor_tensor(out=m2[:], in0=aivsl, in1=BJ,
                                        op=ALU.mult)
                nc.gpsimd.tensor_tensor(out=m1[:], in0=vsl, in1=BJ,
                                        op=ALU.mult)
                nc.vector.tensor_tensor(out=m1[:], in0=m1[:], in1=aivsl,
                                        op=ALU.add)
                for q, mw in ((1, m1), (2, m2)):
                    mb = bass.AP(mw.tensor, mw[:].offset,
                                 [mw[:].ap[0], [0, 4], [1, JC]])
                    dst = lt[:, 4 * q * JC:(4 * q + 4) * JC].rearrange(
                        "p (d j) -> p d j", d=4)
                    src = lt[:, 0:4 * JC].rearrange("p (d j) -> p d j", d=4)
                    nc.vector.tensor_tensor(out=dst, in0=src, in1=mb,
                                            op=ALU.mult)

                # basis planes (fp16): [v, s2, relu(s2 - t_k)]
                bas = bpp.tile([128, WB * JC], F16, tag="bas")
                nc.scalar.copy(bas[:, 0:JC], vsl)
                nc.vector.tensor_copy(out=bas[:, JC:2 * JC], in_=s2)
                for k, t in enumerate(KNOTS):
                    nc.scalar.activation(bas[:, (2 + k) * JC:(3 + k) * JC],
                                         s2, AF.Relu, bias=float(-t))

                # ---- moments + Q + D per 4-group batch (128 atoms) ----
                for b in range(JC // 64):          # 8 batches per chunk
                    # phi_a[32*gg + beta, 12*jj + 4*m + d] per parity half
                    phi_e = psp.tile([128, 192], F32, tag="phie")
                    phi_o = psp.tile([128, 192], F32, tag="phio")
                    phab = (phi_e, phi_o)
                    for gg in range(4):
                        for jj in range(16):
                            j = b * 64 + gg * 16 + jj
                            lhsTs = [bas[0:64, j::JC], bas[64:128, j::JC]]
                            rj = lt[:, j:12 * JC:JC]
                            for a in range(2):
                                nc.tensor.matmul(
                                    out=phab[a][32 * gg:32 * gg + WB,
                                               12 * jj:12 * jj + 12],
                                    lhsT=lhsTs[a],
                                    rhs=rj[64 * a:64 * (a + 1), :],
                                    start=True, stop=True,
                                    tile_position=(64 * a, 32 * gg))
                    # interleave parities: phis[., 24*jj + 12*a + 4*m + d]
                    phis = grpp.tile([128, 384], F16, tag="phis")
                    ph = phis[:]
                    for a, pha in ((0, phi_e), (1, phi_o)):
                        dstv = bass.AP(ph.tensor, ph.offset + 12 * a,
                                       [ph.ap[0], [24, 16], [1, 12]])
                        nc.scalar.copy(dstv, pha[:])

                    q2p = qsp.tile([128, 128], F32, tag="q2p")
                    for gg in range(4):
                        pb = phis[32 * gg:32 * gg + WB, :]
                        for d in range(4):
                            for m in range(3):
                                lw = bass.AP(pb.tensor, pb.offset + 4 * m + d,
                                             [pb.ap[0], [12, 32]])
                                nc.tensor.matmul(
                                    out=q2p[32 * gg:32 * gg + 32,
                                            32 * d:32 * d + 32],
                                    lhsT=lw,
                                    rhs=w3s[32 * gg:32 * gg + WB,
                                            32 * m:32 * m + 32],
                                    start=(m == 0), stop=(m == 2),
                                    tile_position=(32 * gg, 32 * gg))
                    q2 = grpp.tile([128, 128], F16, tag="q2")
                    nc.scalar.copy(q2[:], q2p[:])

                    # D[n, 16k+g] = sum_d Q[d, g] * Q[d, k]  (host transposes)
                    q2a = q2[:]
                    tmp = grpp.tile([128, 4 * 512], F16, tag="tmp")
                    in0 = bass.AP(q2a.tensor, q2a.offset,
                                  [q2a.ap[0], [32, 4], [0, 16], [1, 32]])
                    in1 = bass.AP(q2a.tensor, q2a.offset,
                                  [q2a.ap[0], [32, 4], [1, 16], [0, 32]])
                    tmpv = tmp[:].rearrange("p (d k g) -> p d k g", d=4, g=32)
                    nc.vector.tensor_tensor(out=tmpv, in0=in0, in1=in1,
                                            op=ALU.mult)
                    ta = tmp[:].rearrange("p (e f) -> p e f", e=2)
                    nc.vector.tensor_tensor(
                        out=ta[:, 0, :].rearrange("p (e f) -> p e f", e=2),
                        in0=ta[:, 0, :].rearrange("p (e f) -> p e f", e=2),
                        in1=ta[:, 1, :].rearrange("p (e f) -> p e f", e=2),
                        op=ALU.add)
                    nc.vector.tensor_tensor(
                        out=stage[:, 512 * b:512 * (b + 1)],
                        in0=tmp[:, 0:512], in1=tmp[:, 512:1024], op=ALU.add)

                dst = dout[1024 * c:1024 * (c + 1)].rearrange(
                    "(b p) f -> p b f", p=128)
                src = stage[:].rearrange("p (b f) -> p b f", b=8)
                nc.sync.dma_start(dst, src)

    nc.compile()
    return nc


# ------------------------------ host glue ----------------------------------

def _prep_core(pos, types, neigh):
    comp = np.empty((4, N), np.float32)
    comp[0], comp[1], comp[2] = pos[:, 0], pos[:, 1], pos[:, 2]
    comp[3] = types.astype(np.float32)
    table = np.empty((128, N), np.float32)
    for p in range(4):
        table[p::4] = comp[p]

    nv = neigh.reshape(JTOT, 2, M)
    nq = np.ascontiguousarray(nv.transpose(1, 2, 0).reshape(128, JTOT))
    valid = (nq >= 0)
    nq_cl = np.maximum(nq, 0).astype(np.int16)

    idxw = np.empty((128, JTOT), np.int16)
    for c in range(NCHUNK):
        blk = nq_cl[:, c * JC:(c + 1) * JC]
        for k in range(NCORES):
            stream = blk[16 * k:16 * k + 16, :].reshape(16 * JC)
            idxw[16 * k:16 * k + 16, c * JC:(c + 1) * JC] = \
                stream.reshape(JC, 16).T

    par = pos.reshape(JTOT, 2, 3)

    def repl(x):  # [2, JTOT] -> [128, JTOT]
        return np.ascontiguousarray(
            np.broadcast_to(x[:, None, :], (2, M, JTOT)).reshape(128, JTOT)
        ).astype(np.float32)

    geo = np.empty((128, 3 * JTOT), np.float32)
    for c3 in range(3):
        gr = repl(par[:, :, c3].T)
        for c in range(NCHUNK):
            geo[:, 3 * c * JC + c3 * JC:3 * c * JC + (c3 + 1) * JC] = \
                gr[:, c * JC:(c + 1) * JC]

    vmask = valid.astype(np.float16)
    ai = repl(types.reshape(JTOT, 2).T.astype(np.float32)).astype(np.float16)
    aiv = (ai * vmask).astype(np.float16)
    auxa = np.empty((128, 3 * JTOT), np.int16)
    auxa[:, 0:JTOT] = idxw
    auxa[:, JTOT:2 * JTOT] = vmask.view(np.int16)
    auxa[:, 2 * JTOT:3 * JTOT] = aiv.view(np.int16)
    return dict(table=table, geo=geo, aux=auxa)


_CACHE = {}


def kernel(**inputs):
    inputs = {k: np.asarray(v) for k, v in inputs.items()}
    ws = {k: inputs[k].astype(np.float64) for k in
          ("es1_w", "es1_b", "es2_w", "es2_b", "fs1_w", "fs1_b", "fs2_w",
           "fs2_b", "en1_w", "en1_b", "en2_w", "en2_b", "en3_w", "en3_b")}
    key = hash(tuple(ws[k].tobytes() for k in sorted(ws)))
    if key not in _CACHE:
        w3f = _fold_w3f(ws)
        nc = _build_program()
        _CACHE[key] = (w3f, nc)
    w3f, nc = _CACHE[key]

    pos = inputs["inputs"].astype(np.float32)
    types = inputs["input_types"].astype(np.int64)
    neigh = inputs["neigh_list"].astype(np.int64)

    in_maps = []
    for s in range(S):
        m = _prep_core(pos[s], types[s], neigh[s])
        m["w3f"] = w3f
        in_maps.append(m)

    res = run_bass_kernel_spmd(nc, in_maps, core_ids=list(range(NCORES)))
    # device layout is [N, 16 k, 32 g]; transpose to [N, 32, 16]
    out = np.stack([np.ascontiguousarray(
        r["dout"].astype(np.float32).reshape(N, 16, 32).transpose(0, 2, 1))
        for r in res.results], 0)
    return out
